# revision 10
# baseline (speedup 1.0000x reference)
"""PointPillarsScatter Trainium2 kernel.

Reference op:
  canvas[b*NY*NX + y*NX + x] = voxel_features[p]        (scatter-set, 64 ch)
  out[:, :64]  = canvas -> [B, 64, NY, NX]
  out[:, 64:]  = transpose(map_fm, (0, 3, 2, 1))        (16 ch)

Strategy (8 NeuronCores, SPMD):
  core = batch*2 + y_half  (4 batches x 2 halves of NY=496 -> NYH=248 rows).
  Scatter is computed as a one-hot matmul on the TensorEngine:
    out_tile[64ch, 512cells] = featT[slots, 64].T @ S[slots, 512]
  where S[s, n] = (pos[s] == n), built on GPSIMD with iota + is_equal.
  This fuses zero-fill + scatter + transpose into a single PE op and makes
  the kernel purely DMA-bound.
  The matmul runs in fp32r mode (1 column/cycle vs 4 for fp32). fp32r is
  e8m11 (lossy), so each feature value v is split exactly into
  v = hi + lo with both parts e8m11-representable (hi = truncate mantissa
  to 11 bits, lo = v - hi has <= 12 significant bits). hi parts occupy
  slots 0..63, lo parts slots 64..127 of the same matmul (S rows
  duplicated), so one K=128 matmul reconstructs v exactly in fp32 PSUM.
  map_fm is transposed with PE transpose (identity) in 128x128 blocks.

Host side only computes index tables + shards inputs (per the sharding
hint: route points by coords to their core), all FP math is on device.
"""

import sys

for _p in ("/opt/trn_rl_repo",):
    if _p not in sys.path:
        sys.path.insert(0, _p)

import numpy as np

# problem constants (hardcoded per contract)
B, NPTS, C, NY, NX, CM = 4, 48000, 64, 496, 432, 16
NYH = NY // 2            # 248 rows per core
NCORE = 8
NCELL = NYH * NX         # 107136 cells per core
TILE = 512               # scatter tile = one PSUM bank of fp32
NT = (NCELL + TILE - 1) // TILE          # 210 (last tile has 128 cells)
CAP = 64                 # point slots per matmul chunk
FB = 16                  # feature-table columns per DMA load
SG = 8                   # scatter tiles per SBUF staging buffer / out DMA
YB = 8                   # map y rows per transpose block ( YB*CM = 128 )
NYB = NYH // YB          # 31 y-blocks
XCH = [(0, 128), (128, 128), (256, 128), (384, 48)]   # x chunks of NX=432
KG = 4                   # y-blocks fused per PSUM bank / map out DMA

_prog_cache = {}


def _build_program(ncols, chunks):
    """Build the SPMD Bass program (identical for all 8 cores)."""
    from concourse import bacc, mybir, tile
    from concourse.masks import make_identity

    f32 = mybir.dt.float32
    f32r = mybir.dt.float32r
    i32 = mybir.dt.int32

    nc = bacc.Bacc(trn_type="TRN2", target_bir_lowering=False)

    feat_d = nc.dram_tensor("feat", [ncols * 2 * CAP, C], f32r,
                            kind="ExternalInput")
    post_d = nc.dram_tensor("post", [2 * CAP, ncols], f32, kind="ExternalInput")
    map_d = nc.dram_tensor("mapin", [NX, NYH, CM], f32, kind="ExternalInput")
    out_d = nc.dram_tensor("out", [C + CM, NCELL], f32, kind="ExternalOutput")

    # column index of each (tile, chunk)
    colbase = np.concatenate([[0], np.cumsum(chunks)]).astype(np.int64)

    # map work groups: (x0, w, yb0, kk) - KG consecutive y-blocks, same x chunk
    map_groups = []
    for x0, w in XCH:
        yb = 0
        while yb < NYB:
            kk = min(KG, NYB - yb)
            map_groups.append((x0, w, yb, kk))
            yb += kk
    mg_iter = iter(map_groups)
    n_sc_groups = (NT + SG - 1) // SG
    # interleave: emit map groups spread across scatter groups
    mg_every = max(1, n_sc_groups // len(map_groups))

    with tile.TileContext(nc) as tc:
        with (
            tc.tile_pool(name="const", bufs=1) as cpool,
            tc.tile_pool(name="fpool", bufs=2) as fpool,
            tc.tile_pool(name="spool", bufs=4) as spool,
            tc.tile_pool(name="stg", bufs=2) as stpool,
            tc.tile_pool(name="mstg", bufs=2) as mstpool,
            tc.tile_pool(name="mtin", bufs=3) as mtpool,
            tc.tile_pool(name="pscat", bufs=4, space="PSUM") as pspool,
            tc.tile_pool(name="pmap", bufs=2, space="PSUM") as pmpool,
        ):
            # constants
            iota_i = cpool.tile([2 * CAP, TILE], i32)
            nc.gpsimd.iota(iota_i[:], pattern=[[1, TILE]], base=0,
                           channel_multiplier=0)
            iota_f = cpool.tile([2 * CAP, TILE], f32)
            nc.gpsimd.tensor_copy(iota_f[:], iota_i[:])
            ident = cpool.tile([128, 128], f32)
            make_identity(nc, ident[:])
            posT = cpool.tile([2 * CAP, ncols], f32)
            nc.sync.dma_start(out=posT[:], in_=post_d[:])

            def emit_map_group(x0, w, yb0, kk):
                pm = pmpool.tile([128, KG * 128], f32)
                for k in range(kk):
                    mt = mtpool.tile([128, YB * CM], f32)
                    src = map_d[x0:x0 + w, (yb0 + k) * YB:(yb0 + k + 1) * YB, :]
                    nc.sync.dma_start(
                        out=mt[:w, :], in_=src.rearrange("x y c -> x (y c)"))
                    nc.tensor.transpose(out=pm[:, k * w:(k + 1) * w],
                                        in_=mt[:w, :], identity=ident[:w, :w])
                ms = mstpool.tile([128, KG * 128], f32)
                nc.vector.tensor_copy(out=ms[:, :kk * w], in_=pm[:, :kk * w])
                # DRAM: channel 64+c, cell (yb0*YB + k*YB + dy)*NX + x0 + x
                full = out_d[C:C + CM, :].rearrange("c (yy xx) -> c yy xx",
                                                    xx=NX)
                for k in range(kk):
                    dst = full[:, (yb0 + k) * YB:(yb0 + k + 1) * YB,
                               x0:x0 + w]
                    nc.sync.dma_start(
                        out=dst.rearrange("c dy x -> dy c x"),
                        in_=ms[:, k * w:(k + 1) * w])

            # scatter loop over groups of SG tiles
            emitted_maps = 0
            for g in range(n_sc_groups):
                t0 = g * SG
                t1 = min(t0 + SG, NT)
                cells0 = t0 * TILE
                gw = min(t1 * TILE, NCELL) - cells0
                # feature loads for the columns of this tile group
                c0 = int(colbase[t0])
                c1 = int(colbase[t1])
                fb = fpool.tile([2 * CAP, FB * C], f32r, tag="fb")
                assert c1 - c0 <= FB, (c0, c1)
                fsrc = feat_d[c0 * 2 * CAP:c1 * 2 * CAP, :]
                nc.sync.dma_start(
                    out=fb[:, :(c1 - c0) * C].rearrange(
                        "s (t c) -> s t c", c=C),
                    in_=fsrc.rearrange("(t s) c -> s t c", s=2 * CAP))
                stg = stpool.tile([C, SG * TILE], f32)
                for t in range(t0, t1):
                    n = min(TILE, NCELL - t * TILE)
                    ps = pspool.tile([C, TILE], f32)
                    nck = int(chunks[t])
                    for k in range(nck):
                        col = int(colbase[t]) + k
                        s_t = spool.tile([2 * CAP, TILE], f32r)
                        nc.gpsimd.tensor_scalar(
                            out=s_t[:, :n], in0=iota_f[:, :n],
                            scalar1=posT[:, col:col + 1], scalar2=None,
                            op0=mybir.AluOpType.is_equal)
                        lhs = fb[:, (col - c0) * C:(col - c0 + 1) * C]
                        nc.tensor.matmul(
                            out=ps[:, :n], lhsT=lhs,
                            rhs=s_t[:, :n],
                            start=(k == 0), stop=(k == nck - 1))
                    off = (t - t0) * TILE
                    nc.vector.tensor_copy(out=stg[:, off:off + n],
                                          in_=ps[:, :n])
                nc.sync.dma_start(out=out_d[0:C, cells0:cells0 + gw],
                                  in_=stg[:, :gw])
                # interleave map groups
                while (emitted_maps < len(map_groups)
                       and emitted_maps <= g // mg_every):
                    emit_map_group(*next(mg_iter))
                    emitted_maps += 1
            for mg in mg_iter:
                emit_map_group(*mg)

    nc.finalize()
    return nc


def _host_prep(voxel_features, coords, map_fm):
    """Shard points by core, build feature/pos tables (host index work only)."""
    vf = np.ascontiguousarray(np.asarray(voxel_features), dtype=np.float32)
    cd = np.asarray(coords)
    mf = np.asarray(map_fm)
    if mf.ndim == 5:
        mf = np.squeeze(mf, 3)
    mf = np.ascontiguousarray(mf, dtype=np.float32)

    b = cd[:, 0].astype(np.int64)
    y = cd[:, 2].astype(np.int64)
    x = cd[:, 3].astype(np.int64)
    valid = (b >= 0) & (b < B) & (y >= 0) & (y < NY) & (x >= 0) & (x < NX)
    b, y, x = b[valid], y[valid], x[valid]
    vfv = vf[valid]

    half = (y >= NYH).astype(np.int64)
    core = b * 2 + half
    lcell = (y - half * NYH) * NX + x
    t = lcell // TILE
    pos = lcell - t * TILE

    key = core * NT + t
    order = np.argsort(key, kind="stable")
    ks = key[order]
    counts = np.bincount(ks, minlength=NCORE * NT)
    kmax = counts.reshape(NCORE, NT).max(axis=0)
    chunks = np.maximum((kmax + CAP - 1) // CAP, 1)
    # feature loads assume all chunks of a SG-tile group fit in FB columns
    for g in range(0, NT, SG):
        need = int(chunks[g:g + SG].sum())
        while need > FB:  # pathological overflow: grow FB would change program
            raise ValueError("tile group needs %d columns > FB=%d" % (need, FB))
    ncols = int(chunks.sum())
    colbase = np.concatenate([[0], np.cumsum(chunks)]).astype(np.int64)

    starts = np.concatenate([[0], np.cumsum(counts)]).astype(np.int64)
    rank = np.arange(len(ks), dtype=np.int64) - starts[ks]

    co = core[order]
    to = t[order]
    colo = colbase[to] + rank // CAP
    slot = rank % CAP

    # exact fp32 = hi + lo split, both parts e8m11 (fp32r) representable:
    # hi = mantissa truncated to 11 bits, lo = residual (<= 12 sig. bits)
    vo = vfv[order]
    hi = (vo.view(np.uint32) & np.uint32(0xFFFFF000)).view(np.float32)
    lo = vo - hi

    feat = np.zeros((NCORE, ncols * 2 * CAP, C), np.float32)
    post = np.full((NCORE, 2 * CAP, ncols), -1.0, np.float32)
    feat[co, colo * 2 * CAP + slot, :] = hi
    feat[co, colo * 2 * CAP + CAP + slot, :] = lo
    posf = pos[order].astype(np.float32)
    post[co, slot, colo] = posf
    post[co, CAP + slot, colo] = posf

    maps = []
    for core_id in range(NCORE):
        bb, hh = core_id // 2, core_id % 2
        maps.append(np.ascontiguousarray(
            mf[bb, :, hh * NYH:(hh + 1) * NYH, :]))
    return feat, post, maps, ncols, chunks


def kernel(voxel_features, coords, batch_size=None, map_fm=None,
           trace=False, _return_results=False):
    from concourse.bass_utils import run_bass_kernel_spmd

    feat, post, maps, ncols, chunks = _host_prep(
        voxel_features, coords, map_fm)

    ckey = (ncols, tuple(int(c) for c in chunks))
    if ckey not in _prog_cache:
        _prog_cache.clear()
        _prog_cache[ckey] = _build_program(ncols, chunks)
    nc = _prog_cache[ckey]

    in_maps = [
        {"feat": feat[i], "post": post[i], "mapin": maps[i]}
        for i in range(NCORE)
    ]
    res = run_bass_kernel_spmd(nc, in_maps, list(range(NCORE)), trace=trace)

    out = np.empty((B, C + CM, NY, NX), np.float32)
    for core_id in range(NCORE):
        bb, hh = core_id // 2, core_id % 2
        out[bb, :, hh * NYH:(hh + 1) * NYH, :] = (
            res.results[core_id]["out"].reshape(C + CM, NYH, NX))
    if _return_results:
        return out, res
    return out


# revision 12
# speedup vs baseline: 4.4109x; 4.4109x over previous
"""PointPillarsScatter Trainium2 kernel.

Reference op:
  canvas[b*NY*NX + y*NX + x] = voxel_features[p]        (scatter-set, 64 ch)
  out[:, :64]  = canvas -> [B, 64, NY, NX]
  out[:, 64:]  = transpose(map_fm, (0, 3, 2, 1))        (16 ch)

Strategy (8 NeuronCores, SPMD):
  core = batch*2 + y_half  (4 batches x 2 halves of NY=496 -> NYH=248 rows).
  Scatter is computed as a one-hot matmul on the TensorEngine:
    out_tile[64ch, 512cells] = featT[slots, 64].T @ S[slots, 512]
  where S[s, n] = (pos[s] == n), built on GPSIMD with iota + is_equal.
  This fuses zero-fill + scatter + transpose into a single PE op and makes
  the kernel purely DMA-bound.
  The matmul runs in fp32r mode (1 column/cycle vs 4 for fp32). fp32r is
  e8m11 (lossy), so each feature value v is split exactly into
  v = hi + lo with both parts e8m11-representable (hi = truncate mantissa
  to 11 bits, lo = v - hi has <= 12 significant bits). hi parts occupy
  slots 0..63, lo parts slots 64..127 of the same matmul (S rows
  duplicated), so one K=128 matmul reconstructs v exactly in fp32 PSUM.
  map_fm is transposed with PE transpose (identity) in 128x128 blocks.

Host side only computes index tables + shards inputs (per the sharding
hint: route points by coords to their core), all FP math is on device.
"""

import sys

for _p in ("/opt/trn_rl_repo",):
    if _p not in sys.path:
        sys.path.insert(0, _p)

import numpy as np

# problem constants (hardcoded per contract)
B, NPTS, C, NY, NX, CM = 4, 48000, 64, 496, 432, 16
NYH = NY // 2            # 248 rows per core
NCORE = 8
NCELL = NYH * NX         # 107136 cells per core
TILE = 512               # scatter tile = one PSUM bank of fp32
NT = (NCELL + TILE - 1) // TILE          # 210 (last tile has 128 cells)
CAP = 64                 # point slots per matmul chunk
FB = 16                  # feature-table columns per DMA load
SG = 8                   # scatter tiles per SBUF staging buffer / out DMA
YB = 8                   # map y rows per transpose block ( YB*CM = 128 )
NYB = NYH // YB          # 31 y-blocks
XCH = [(0, 128), (128, 128), (256, 128), (384, 48)]   # x chunks of NX=432
KG = 4                   # y-blocks fused per PSUM bank / map out DMA

_prog_cache = {}


def _build_program(ncols, chunks):
    """Build the SPMD Bass program (identical for all 8 cores)."""
    from concourse import bacc, mybir, tile
    from concourse.masks import make_identity

    f32 = mybir.dt.float32
    f32r = mybir.dt.float32r
    i32 = mybir.dt.int32

    nc = bacc.Bacc(trn_type="TRN2", target_bir_lowering=False)

    feat_d = nc.dram_tensor("feat", [ncols * 2 * CAP, C], f32r,
                            kind="ExternalInput")
    post_d = nc.dram_tensor("post", [2 * CAP, ncols], f32, kind="ExternalInput")
    map_d = nc.dram_tensor("mapin", [NX, NYH, CM], f32, kind="ExternalInput")
    out_d = nc.dram_tensor("out", [C + CM, NCELL], f32, kind="ExternalOutput")

    # column index of each (tile, chunk)
    colbase = np.concatenate([[0], np.cumsum(chunks)]).astype(np.int64)

    # map work groups: (x0, w, yb0, kk) - KG consecutive y-blocks, same x chunk
    map_groups = []
    for x0, w in XCH:
        yb = 0
        while yb < NYB:
            kk = min(KG, NYB - yb)
            map_groups.append((x0, w, yb, kk))
            yb += kk
    mg_iter = iter(map_groups)
    n_sc_groups = (NT + SG - 1) // SG
    # interleave: emit map groups spread across scatter groups
    mg_every = max(1, n_sc_groups // len(map_groups))

    with tile.TileContext(nc) as tc:
        with (
            tc.tile_pool(name="const", bufs=1) as cpool,
            tc.tile_pool(name="fpool", bufs=2) as fpool,
            tc.tile_pool(name="spool", bufs=4) as spool,
            tc.tile_pool(name="stg", bufs=2) as stpool,
            tc.tile_pool(name="mstg", bufs=2) as mstpool,
            tc.tile_pool(name="mtin", bufs=3) as mtpool,
            tc.tile_pool(name="pscat", bufs=4, space="PSUM") as pspool,
            tc.tile_pool(name="pmap", bufs=2, space="PSUM") as pmpool,
        ):
            # constants
            iota_i = cpool.tile([2 * CAP, TILE], i32)
            nc.gpsimd.iota(iota_i[:], pattern=[[1, TILE]], base=0,
                           channel_multiplier=0)
            iota_f = cpool.tile([2 * CAP, TILE], f32)
            nc.gpsimd.tensor_copy(iota_f[:], iota_i[:])
            ident = cpool.tile([128, 128], f32)
            make_identity(nc, ident[:])
            posT = cpool.tile([2 * CAP, ncols], f32)
            nc.sync.dma_start(out=posT[:], in_=post_d[:])

            def emit_map_group(x0, w, yb0, kk):
                pm = pmpool.tile([128, KG * 128], f32)
                for k in range(kk):
                    mt = mtpool.tile([128, YB * CM], f32)
                    src = map_d[x0:x0 + w, (yb0 + k) * YB:(yb0 + k + 1) * YB, :]
                    nc.sync.dma_start(
                        out=mt[:w, :], in_=src.rearrange("x y c -> x (y c)"))
                    nc.tensor.transpose(out=pm[:, k * w:(k + 1) * w],
                                        in_=mt[:w, :], identity=ident[:w, :w])
                ms = mstpool.tile([128, KG * 128], f32)
                nc.scalar.copy(out=ms[:, :kk * w], in_=pm[:, :kk * w])
                # DRAM: channel 64+c, cell (yb0*YB + k*YB + dy)*NX + x0 + x
                full = out_d[C:C + CM, :].rearrange("c (yy xx) -> c yy xx",
                                                    xx=NX)
                for k in range(kk):
                    dst = full[:, (yb0 + k) * YB:(yb0 + k + 1) * YB,
                               x0:x0 + w]
                    nc.sync.dma_start(
                        out=dst.rearrange("c dy x -> dy c x"),
                        in_=ms[:, k * w:(k + 1) * w])

            # scatter loop over groups of SG tiles
            emitted_maps = 0
            for g in range(n_sc_groups):
                t0 = g * SG
                t1 = min(t0 + SG, NT)
                cells0 = t0 * TILE
                gw = min(t1 * TILE, NCELL) - cells0
                # feature loads for the columns of this tile group
                c0 = int(colbase[t0])
                c1 = int(colbase[t1])
                fb = fpool.tile([2 * CAP, FB * C], f32r, tag="fb")
                assert c1 - c0 <= FB, (c0, c1)
                fsrc = feat_d[c0 * 2 * CAP:c1 * 2 * CAP, :]
                nc.sync.dma_start(
                    out=fb[:, :(c1 - c0) * C].rearrange(
                        "s (t c) -> s t c", c=C),
                    in_=fsrc.rearrange("(t s) c -> s t c", s=2 * CAP))
                stg = stpool.tile([C, SG * TILE], f32)
                for t in range(t0, t1):
                    n = min(TILE, NCELL - t * TILE)
                    ps = pspool.tile([C, TILE], f32)
                    nck = int(chunks[t])
                    for k in range(nck):
                        col = int(colbase[t]) + k
                        s_t = spool.tile([2 * CAP, TILE], f32r)
                        nc.vector.tensor_scalar(
                            out=s_t[:, :n], in0=iota_f[:, :n],
                            scalar1=posT[:, col:col + 1], scalar2=None,
                            op0=mybir.AluOpType.is_equal)
                        lhs = fb[:, (col - c0) * C:(col - c0 + 1) * C]
                        nc.tensor.matmul(
                            out=ps[:, :n], lhsT=lhs,
                            rhs=s_t[:, :n],
                            start=(k == 0), stop=(k == nck - 1))
                    off = (t - t0) * TILE
                    # split PSUM evacuation between DVE and ACT
                    eng = nc.vector.tensor_copy if t % 2 == 0 else (
                        lambda out, in_: nc.scalar.copy(out=out, in_=in_))
                    eng(out=stg[:, off:off + n], in_=ps[:, :n])
                nc.sync.dma_start(out=out_d[0:C, cells0:cells0 + gw],
                                  in_=stg[:, :gw])
                # interleave map groups
                while (emitted_maps < len(map_groups)
                       and emitted_maps <= g // mg_every):
                    emit_map_group(*next(mg_iter))
                    emitted_maps += 1
            for mg in mg_iter:
                emit_map_group(*mg)

    nc.finalize()
    return nc


def _host_prep(voxel_features, coords, map_fm):
    """Shard points by core, build feature/pos tables (host index work only)."""
    vf = np.ascontiguousarray(np.asarray(voxel_features), dtype=np.float32)
    cd = np.asarray(coords)
    mf = np.asarray(map_fm)
    if mf.ndim == 5:
        mf = np.squeeze(mf, 3)
    mf = np.ascontiguousarray(mf, dtype=np.float32)

    b = cd[:, 0].astype(np.int64)
    y = cd[:, 2].astype(np.int64)
    x = cd[:, 3].astype(np.int64)
    valid = (b >= 0) & (b < B) & (y >= 0) & (y < NY) & (x >= 0) & (x < NX)
    b, y, x = b[valid], y[valid], x[valid]
    vfv = vf[valid]

    half = (y >= NYH).astype(np.int64)
    core = b * 2 + half
    lcell = (y - half * NYH) * NX + x
    t = lcell // TILE
    pos = lcell - t * TILE

    key = core * NT + t
    order = np.argsort(key, kind="stable")
    ks = key[order]
    counts = np.bincount(ks, minlength=NCORE * NT)
    kmax = counts.reshape(NCORE, NT).max(axis=0)
    chunks = np.maximum((kmax + CAP - 1) // CAP, 1)
    # feature loads assume all chunks of a SG-tile group fit in FB columns
    for g in range(0, NT, SG):
        need = int(chunks[g:g + SG].sum())
        while need > FB:  # pathological overflow: grow FB would change program
            raise ValueError("tile group needs %d columns > FB=%d" % (need, FB))
    ncols = int(chunks.sum())
    colbase = np.concatenate([[0], np.cumsum(chunks)]).astype(np.int64)

    starts = np.concatenate([[0], np.cumsum(counts)]).astype(np.int64)
    rank = np.arange(len(ks), dtype=np.int64) - starts[ks]

    co = core[order]
    to = t[order]
    colo = colbase[to] + rank // CAP
    slot = rank % CAP

    # exact fp32 = hi + lo split, both parts e8m11 (fp32r) representable:
    # hi = mantissa truncated to 11 bits, lo = residual (<= 12 sig. bits)
    vo = vfv[order]
    hi = (vo.view(np.uint32) & np.uint32(0xFFFFF000)).view(np.float32)
    lo = vo - hi

    feat = np.zeros((NCORE, ncols * 2 * CAP, C), np.float32)
    post = np.full((NCORE, 2 * CAP, ncols), -1.0, np.float32)
    feat[co, colo * 2 * CAP + slot, :] = hi
    feat[co, colo * 2 * CAP + CAP + slot, :] = lo
    posf = pos[order].astype(np.float32)
    post[co, slot, colo] = posf
    post[co, CAP + slot, colo] = posf

    maps = []
    for core_id in range(NCORE):
        bb, hh = core_id // 2, core_id % 2
        maps.append(np.ascontiguousarray(
            mf[bb, :, hh * NYH:(hh + 1) * NYH, :]))
    return feat, post, maps, ncols, chunks


def kernel(voxel_features, coords, batch_size=None, map_fm=None,
           trace=False, _return_results=False):
    from concourse.bass_utils import run_bass_kernel_spmd

    feat, post, maps, ncols, chunks = _host_prep(
        voxel_features, coords, map_fm)

    ckey = (ncols, tuple(int(c) for c in chunks))
    if ckey not in _prog_cache:
        _prog_cache.clear()
        _prog_cache[ckey] = _build_program(ncols, chunks)
    nc = _prog_cache[ckey]

    in_maps = [
        {"feat": feat[i], "post": post[i], "mapin": maps[i]}
        for i in range(NCORE)
    ]
    res = run_bass_kernel_spmd(nc, in_maps, list(range(NCORE)), trace=trace)

    out = np.empty((B, C + CM, NY, NX), np.float32)
    for core_id in range(NCORE):
        bb, hh = core_id // 2, core_id % 2
        out[bb, :, hh * NYH:(hh + 1) * NYH, :] = (
            res.results[core_id]["out"].reshape(C + CM, NYH, NX))
    if _return_results:
        return out, res
    return out


# revision 18
# speedup vs baseline: 6.7466x; 1.5295x over previous
"""PointPillarsScatter Trainium2 kernel.

Reference op:
  canvas[b*NY*NX + y*NX + x] = voxel_features[p]        (scatter-set, 64 ch)
  out[:, :64]  = canvas -> [B, 64, NY, NX]
  out[:, 64:]  = transpose(map_fm, (0, 3, 2, 1))        (16 ch)

Strategy (8 NeuronCores, SPMD):
  core = batch*2 + y_half  (4 batches x 2 halves of NY=496 -> NYH=248 rows).

  Scatter is computed as a one-hot matmul on the TensorEngine:
    out[128ch', 512cells] = featT[128slots, 128ch'].T @ S[128slots, 512]
  where S[s, n] = (pos[s] == n) is built on the VectorEngine with
  iota + is_equal, and ch' packs the 64 channels of TWO 512-cell tiles
  (tile j -> psum partitions 0:64, tile j+105 -> 64:128, so each
  partition half maps to a contiguous DRAM range).  This fuses
  zero-fill + scatter + transpose into one PE op per 1024 cells.

  The matmul runs in fp32r mode (1 column/cycle vs 4 for fp32). fp32r is
  e8m11 (lossy), so each feature value v is split exactly into
  v = hi + lo, both parts e8m11-representable (hi = mantissa truncated
  to 11 bits, lo = v - hi has <= 12 significant bits). hi parts occupy
  slots 0..63, lo parts slots 64..127 with identical S rows, so a single
  K=128 matmul reconstructs v exactly in fp32 PSUM.

  map_fm is transposed with PE transpose (identity) in [<=128, 128]
  blocks, staged per output y-row so every DMA moves 1728B contiguous
  runs.

Host side only computes index tables + shards inputs (per the sharding
hint: route points by coords to their core); all FP math runs on device.
"""

import sys

for _p in ("/opt/trn_rl_repo",):
    if _p not in sys.path:
        sys.path.insert(0, _p)

import numpy as np

# problem constants (hardcoded per contract)
B, NPTS, C, NY, NX, CM = 4, 48000, 64, 496, 432, 16
NYH = NY // 2            # 248 rows per core
NCORE = 8
NCELL = NYH * NX         # 107136 cells per core
TILE = 512               # cells per channel-block
PAIR = 2 * TILE          # cells per matmul (two 64ch blocks -> M=128)
NT = (NCELL + TILE - 1) // TILE          # 210 tiles (last has 128 cells)
NP = (NT + 1) // 2                       # 105 pairs: tile j with tile j+NP
ACELL = NP * TILE                        # 53760 cells in the A half
BCELL = NCELL - ACELL                    # 53376 cells in the B half
CAP = 64                 # points per pair-column (x2 slots for hi/lo)
FB = 8                   # pair-columns per feature DMA load
SG = 4                   # pairs per SBUF staging buffer / out DMA
YB = 8                   # map y rows per transpose block ( YB*CM = 128 )
NYB = NYH // YB          # 31 y-blocks
XCH = [(0, 128), (128, 128), (256, 128), (384, 48)]   # x chunks of NX=432
KG = 4                   # y-blocks per map in-load batch

_prog_cache = {}


def _build_program(ncols, chunks):
    """Build the SPMD Bass program (identical for all 8 cores)."""
    from concourse import bacc, mybir, tile
    from concourse.masks import make_identity

    f32 = mybir.dt.float32
    f32r = mybir.dt.float32r
    i32 = mybir.dt.int32

    nc = bacc.Bacc(trn_type="TRN2", target_bir_lowering=False)

    feat_d = nc.dram_tensor("feat", [ncols * 2 * CAP, 2 * C], f32r,
                            kind="ExternalInput")
    post_d = nc.dram_tensor("post", [2 * CAP, ncols], f32, kind="ExternalInput")
    map_d = nc.dram_tensor("mapin", [NX, NYH, CM], f32, kind="ExternalInput")
    out_d = nc.dram_tensor("out", [C + CM, NCELL], f32, kind="ExternalOutput")

    # column index of each (pair, chunk)
    colbase = np.concatenate([[0], np.cumsum(chunks)]).astype(np.int64)

    # map work: groups of KG y-blocks; 4 x-chunk loads per group
    map_groups = []
    yb = 0
    while yb < NYB:
        kk = min(KG, NYB - yb)
        map_groups.append((yb, kk))
        yb += kk
    mg_iter = iter(map_groups)
    n_sc_groups = (NP + SG - 1) // SG
    mg_every = max(1, n_sc_groups // len(map_groups))

    with tile.TileContext(nc) as tc:
        with (
            tc.tile_pool(name="const", bufs=1) as cpool,
            tc.tile_pool(name="fpool", bufs=2) as fpool,
            tc.tile_pool(name="spool", bufs=4) as spool,
            tc.tile_pool(name="stg", bufs=2) as stpool,
            tc.tile_pool(name="mstg", bufs=3) as mstpool,
            tc.tile_pool(name="mtin", bufs=8) as mtpool,
            tc.tile_pool(name="pscat", bufs=4, space="PSUM") as pspool,
            tc.tile_pool(name="pmap", bufs=3, space="PSUM") as pmpool,
        ):
            # constants
            iota_i = cpool.tile([2 * CAP, TILE], i32)
            nc.gpsimd.iota(iota_i[:], pattern=[[1, TILE]], base=0,
                           channel_multiplier=0)
            iota_f = cpool.tile([2 * CAP, TILE], f32)
            nc.gpsimd.tensor_copy(iota_f[:], iota_i[:])
            ident = cpool.tile([128, 128], f32)
            make_identity(nc, ident[:])
            posT = cpool.tile([2 * CAP, ncols], f32)
            nc.sync.dma_start(out=posT[:], in_=post_d[:])

            def emit_map_group(yb0, kk):
                # in-loads: one per x chunk, kk y-blocks each (2KB runs)
                mts = []
                for x0, w in XCH:
                    mt = mtpool.tile([128, KG * YB * CM], f32, tag="mt")
                    src = map_d[x0:x0 + w, yb0 * YB:(yb0 + kk) * YB, :]
                    nc.scalar.dma_start(
                        out=mt[:w, :kk * YB * CM],
                        in_=src.rearrange("x y c -> x (y c)"))
                    mts.append(mt)
                full = out_d[C:C + CM, :].rearrange("c (yy xx) -> c yy xx",
                                                    xx=NX)
                for k in range(kk):
                    pm = pmpool.tile([128, NX], f32)
                    for xi, (x0, w) in enumerate(XCH):
                        nc.tensor.transpose(
                            out=pm[:, x0:x0 + w],
                            in_=mts[xi][:w, (k * YB * CM):((k + 1) * YB * CM)],
                            identity=ident[:w, :w])
                    ms = mstpool.tile([128, NX], f32)
                    nc.scalar.copy(out=ms[:], in_=pm[:])
                    # one DMA per y-block: runs of NX*4 = 1728B
                    dst = full[:, (yb0 + k) * YB:(yb0 + k + 1) * YB, :]
                    nc.scalar.dma_start(
                        out=dst.rearrange("c dy x -> dy c x"), in_=ms[:])

            # scatter loop over groups of SG pairs
            emitted_maps = 0
            for g in range(n_sc_groups):
                p0 = g * SG
                p1 = min(p0 + SG, NP)
                c0 = int(colbase[p0])
                c1 = int(colbase[p1])
                fb = fpool.tile([2 * CAP, FB * 2 * C], f32r, tag="fb")
                assert c1 - c0 <= FB, (c0, c1)
                fsrc = feat_d[c0 * 2 * CAP:c1 * 2 * CAP, :]
                nc.sync.dma_start(
                    out=fb[:, :(c1 - c0) * 2 * C].rearrange(
                        "s (t c) -> s t c", c=2 * C),
                    in_=fsrc.rearrange("(t s) c -> s t c", s=2 * CAP))
                stg = stpool.tile([128, SG * TILE], f32)
                for pr in range(p0, p1):
                    # block A = tile pr (always 512 cells),
                    # block B = tile NP+pr (last one has 128 cells)
                    nbb = min(TILE, max(0, NCELL - (NP + pr) * TILE))
                    ps = pspool.tile([128, TILE], f32)
                    nck = int(chunks[pr])
                    for k in range(nck):
                        col = int(colbase[pr]) + k
                        s_t = spool.tile([2 * CAP, TILE], f32r)
                        nc.vector.tensor_scalar(
                            out=s_t[:], in0=iota_f[:],
                            scalar1=posT[:, col:col + 1], scalar2=None,
                            op0=mybir.AluOpType.is_equal)
                        lhs = fb[:, (col - c0) * 2 * C:(col - c0 + 1) * 2 * C]
                        nc.tensor.matmul(
                            out=ps[:], lhsT=lhs,
                            rhs=s_t[:],
                            start=(k == 0), stop=(k == nck - 1))
                    off = (pr - p0) * TILE
                    eng = nc.vector.tensor_copy if pr % 2 == 0 else (
                        lambda out, in_: nc.scalar.copy(out=out, in_=in_))
                    eng(out=stg[:, off:off + TILE], in_=ps[:])
                # two out DMAs: A half + B half, each contiguous (<=8KB runs)
                wa = (p1 - p0) * TILE
                a0 = p0 * TILE
                nc.sync.dma_start(out=out_d[0:C, a0:a0 + wa],
                                  in_=stg[:64, :wa])
                wb = min(BCELL - a0, wa)
                if wb > 0:
                    nc.sync.dma_start(
                        out=out_d[0:C, ACELL + a0:ACELL + a0 + wb],
                        in_=stg[64:, :wb])
                while (emitted_maps < len(map_groups)
                       and emitted_maps <= g // mg_every):
                    emit_map_group(*next(mg_iter))
                    emitted_maps += 1
            for mg in mg_iter:
                emit_map_group(*mg)

    nc.finalize()
    return nc


def _host_prep(voxel_features, coords, map_fm):
    """Shard points by core, build feature/pos tables (host index work only)."""
    vf = np.ascontiguousarray(np.asarray(voxel_features), dtype=np.float32)
    cd = np.asarray(coords)
    mf = np.asarray(map_fm)
    if mf.ndim == 5:
        mf = np.squeeze(mf, 3)
    mf = np.ascontiguousarray(mf, dtype=np.float32)

    b = cd[:, 0].astype(np.int64)
    y = cd[:, 2].astype(np.int64)
    x = cd[:, 3].astype(np.int64)
    valid = (b >= 0) & (b < B) & (y >= 0) & (y < NY) & (x >= 0) & (x < NX)
    b, y, x = b[valid], y[valid], x[valid]
    vfv = vf[valid]

    half = (y >= NYH).astype(np.int64)
    core = b * 2 + half
    lcell = (y - half * NYH) * NX + x
    t = lcell // TILE          # 512-cell tile id
    pos = lcell - t * TILE     # position within tile (= matmul column)
    pair = t % NP              # tile j pairs with tile j+NP
    blk = t // NP              # channel block within the pair

    key = core * NP + pair
    order = np.argsort(key, kind="stable")
    ks = key[order]
    counts = np.bincount(ks, minlength=NCORE * NP)
    kmax = counts.reshape(NCORE, NP).max(axis=0)
    chunks = np.maximum((kmax + CAP - 1) // CAP, 1)
    for g in range(0, NP, SG):
        need = int(chunks[g:g + SG].sum())
        if need > FB:
            raise ValueError("pair group needs %d cols > FB=%d" % (need, FB))
    ncols = int(chunks.sum())
    colbase = np.concatenate([[0], np.cumsum(chunks)]).astype(np.int64)

    starts = np.concatenate([[0], np.cumsum(counts)]).astype(np.int64)
    rank = np.arange(len(ks), dtype=np.int64) - starts[ks]

    co = core[order]
    po = pair[order]
    bo = blk[order]
    colo = colbase[po] + rank // CAP
    slot = rank % CAP

    # exact fp32 = hi + lo split, both parts e8m11 (fp32r) representable
    vo = vfv[order]
    hi = (vo.view(np.uint32) & np.uint32(0xFFFFF000)).view(np.float32)
    lo = vo - hi

    feat = np.zeros((NCORE, ncols * 2 * CAP, 2 * C), np.float32)
    post = np.full((NCORE, 2 * CAP, ncols), -1.0, np.float32)
    rows = colo * 2 * CAP + slot
    ccol = bo[:, None] * C + np.arange(C)[None, :]
    feat[co[:, None], rows[:, None], ccol] = hi
    feat[co[:, None], (rows + CAP)[:, None], ccol] = lo
    posf = pos[order].astype(np.float32)
    post[co, slot, colo] = posf
    post[co, CAP + slot, colo] = posf

    maps = []
    for core_id in range(NCORE):
        bb, hh = core_id // 2, core_id % 2
        maps.append(np.ascontiguousarray(
            mf[bb, :, hh * NYH:(hh + 1) * NYH, :]))
    return feat, post, maps, ncols, chunks


def kernel(voxel_features, coords, batch_size=None, map_fm=None,
           trace=False, _return_results=False):
    from concourse.bass_utils import run_bass_kernel_spmd

    feat, post, maps, ncols, chunks = _host_prep(
        voxel_features, coords, map_fm)

    ckey = (ncols, tuple(int(c) for c in chunks))
    if ckey not in _prog_cache:
        _prog_cache.clear()
        _prog_cache[ckey] = _build_program(ncols, chunks)
    nc = _prog_cache[ckey]

    in_maps = [
        {"feat": feat[i], "post": post[i], "mapin": maps[i]}
        for i in range(NCORE)
    ]
    res = run_bass_kernel_spmd(nc, in_maps, list(range(NCORE)), trace=trace)

    out = np.empty((B, C + CM, NY, NX), np.float32)
    for core_id in range(NCORE):
        bb, hh = core_id // 2, core_id % 2
        out[bb, :, hh * NYH:(hh + 1) * NYH, :] = (
            res.results[core_id]["out"].reshape(C + CM, NYH, NX))
    if _return_results:
        return out, res
    return out


# revision 24
# speedup vs baseline: 6.9312x; 1.0274x over previous
"""PointPillarsScatter Trainium2 kernel.

Reference op:
  canvas[b*NY*NX + y*NX + x] = voxel_features[p]        (scatter-set, 64 ch)
  out[:, :64]  = canvas -> [B, 64, NY, NX]
  out[:, 64:]  = transpose(map_fm, (0, 3, 2, 1))        (16 ch)

Strategy (8 NeuronCores, SPMD):
  core = batch*2 + y_half  (4 batches x 2 halves of NY=496 -> NYH=248 rows).

  Scatter is computed as a one-hot matmul on the TensorEngine:
    out[128ch', 512cells] = featT[128slots, 128ch'].T @ S[128slots, 512]
  where S[s, n] = (pos[s] == n) is built on the VectorEngine with
  iota + is_equal, and ch' packs the 64 channels of TWO 512-cell tiles
  (tile j -> psum partitions 0:64, tile j+105 -> 64:128, so each
  partition half maps to a contiguous DRAM range).  This fuses
  zero-fill + scatter + transpose into one PE op per 1024 cells.

  The matmul runs in fp32r mode (1 column/cycle vs 4 for fp32). fp32r is
  e8m11 (lossy), so each feature value v is split exactly into
  v = hi + lo, both parts e8m11-representable (hi = mantissa truncated
  to 11 bits, lo = v - hi has <= 12 significant bits). hi parts occupy
  slots 0..63, lo parts slots 64..127 with identical S rows, so a single
  K=128 matmul reconstructs v exactly in fp32 PSUM.

  map_fm is transposed with PE transpose (identity) in [<=128, 128]
  blocks, staged per output y-row so every DMA moves 1728B contiguous
  runs.

Host side only computes index tables + shards inputs (per the sharding
hint: route points by coords to their core); all FP math runs on device.
"""

import sys

for _p in ("/opt/trn_rl_repo",):
    if _p not in sys.path:
        sys.path.insert(0, _p)

import numpy as np

# problem constants (hardcoded per contract)
B, NPTS, C, NY, NX, CM = 4, 48000, 64, 496, 432, 16
NYH = NY // 2            # 248 rows per core
NCORE = 8
NCELL = NYH * NX         # 107136 cells per core
TILE = 512               # cells per channel-block
PAIR = 2 * TILE          # cells per matmul (two 64ch blocks -> M=128)
NT = (NCELL + TILE - 1) // TILE          # 210 tiles (last has 128 cells)
NP = (NT + 1) // 2                       # 105 pairs: tile j with tile j+NP
ACELL = NP * TILE                        # 53760 cells in the A half
BCELL = NCELL - ACELL                    # 53376 cells in the B half
CAP = 64                 # points per pair-column (x2 slots for hi/lo)
FB = 8                   # pair-columns per feature DMA load
SG = 4                   # pairs per SBUF staging buffer / out DMA
YB = 8                   # map y rows per transpose block ( YB*CM = 128 )
NYB = NYH // YB          # 31 y-blocks
XCH = [(0, 128), (128, 128), (256, 128), (384, 48)]   # x chunks of NX=432
KG = 8                   # y-blocks per map in-load batch

_prog_cache = {}


def _build_program(ncols, chunks):
    """Build the SPMD Bass program (identical for all 8 cores)."""
    from concourse import bacc, mybir, tile
    from concourse.masks import make_identity

    f32 = mybir.dt.float32
    f32r = mybir.dt.float32r
    i32 = mybir.dt.int32

    nc = bacc.Bacc(trn_type="TRN2", target_bir_lowering=False)

    # slot-major layout: partition s reads one contiguous 4KB run per load
    feat_d = nc.dram_tensor("feat", [2 * CAP, ncols * 2 * C], f32r,
                            kind="ExternalInput")
    post_d = nc.dram_tensor("post", [2 * CAP, ncols], f32, kind="ExternalInput")
    map_d = nc.dram_tensor("mapin", [NX, NYH, CM], f32, kind="ExternalInput")
    out_d = nc.dram_tensor("out", [C + CM, NCELL], f32, kind="ExternalOutput")

    # column index of each (pair, chunk)
    colbase = np.concatenate([[0], np.cumsum(chunks)]).astype(np.int64)

    # map work: groups of KG y-blocks; 4 x-chunk loads per group
    map_groups = []
    yb = 0
    while yb < NYB:
        kk = min(KG, NYB - yb)
        map_groups.append((yb, kk))
        yb += kk
    mg_iter = iter(map_groups)
    n_sc_groups = (NP + SG - 1) // SG
    mg_every = max(1, n_sc_groups // len(map_groups))

    with tile.TileContext(nc) as tc:
        with (
            tc.tile_pool(name="const", bufs=1) as cpool,
            tc.tile_pool(name="fpool", bufs=2) as fpool,
            tc.tile_pool(name="spool", bufs=4) as spool,
            tc.tile_pool(name="stg", bufs=2) as stpool,
            tc.tile_pool(name="mstg", bufs=3) as mstpool,
            tc.tile_pool(name="mtin", bufs=8) as mtpool,
            tc.tile_pool(name="pscat", bufs=4, space="PSUM") as pspool,
            tc.tile_pool(name="pmap", bufs=3, space="PSUM") as pmpool,
        ):
            # constants
            iota_i = cpool.tile([2 * CAP, TILE], i32)
            nc.gpsimd.iota(iota_i[:], pattern=[[1, TILE]], base=0,
                           channel_multiplier=0)
            iota_f = cpool.tile([2 * CAP, TILE], f32)
            nc.gpsimd.tensor_copy(iota_f[:], iota_i[:])
            ident = cpool.tile([128, 128], f32)
            make_identity(nc, ident[:])
            posT = cpool.tile([2 * CAP, ncols], f32)
            nc.sync.dma_start(out=posT[:], in_=post_d[:])

            def emit_map_group(yb0, kk):
                # in-loads: one per x chunk, kk y-blocks each (2KB runs)
                mts = []
                for x0, w in XCH:
                    mt = mtpool.tile([128, KG * YB * CM], f32, tag="mt")
                    src = map_d[x0:x0 + w, yb0 * YB:(yb0 + kk) * YB, :]
                    nc.scalar.dma_start(
                        out=mt[:w, :kk * YB * CM],
                        in_=src.rearrange("x y c -> x (y c)"))
                    mts.append(mt)
                full = out_d[C:C + CM, :].rearrange("c (yy xx) -> c yy xx",
                                                    xx=NX)
                for k in range(kk):
                    pm = pmpool.tile([128, NX], f32)
                    for xi, (x0, w) in enumerate(XCH):
                        nc.tensor.transpose(
                            out=pm[:, x0:x0 + w],
                            in_=mts[xi][:w, (k * YB * CM):((k + 1) * YB * CM)],
                            identity=ident[:w, :w])
                    ms = mstpool.tile([128, NX], f32)
                    nc.scalar.copy(out=ms[:], in_=pm[:])
                    # one DMA per y-block: runs of NX*4 = 1728B
                    dst = full[:, (yb0 + k) * YB:(yb0 + k + 1) * YB, :]
                    nc.scalar.dma_start(
                        out=dst.rearrange("c dy x -> dy c x"), in_=ms[:])

            # scatter loop over groups of SG pairs
            emitted_maps = 0
            for g in range(n_sc_groups):
                p0 = g * SG
                p1 = min(p0 + SG, NP)
                c0 = int(colbase[p0])
                c1 = int(colbase[p1])
                fb = fpool.tile([2 * CAP, FB * 2 * C], f32r, tag="fb")
                assert c1 - c0 <= FB, (c0, c1)
                nc.sync.dma_start(
                    out=fb[:, :(c1 - c0) * 2 * C],
                    in_=feat_d[:, c0 * 2 * C:c1 * 2 * C])
                stg = stpool.tile([128, SG * TILE], f32)
                for pr in range(p0, p1):
                    # block A = tile pr (always 512 cells),
                    # block B = tile NP+pr (last one has 128 cells)
                    nbb = min(TILE, max(0, NCELL - (NP + pr) * TILE))
                    ps = pspool.tile([128, TILE], f32)
                    nck = int(chunks[pr])
                    for k in range(nck):
                        col = int(colbase[pr]) + k
                        s_t = spool.tile([2 * CAP, TILE], f32r)
                        nc.vector.tensor_scalar(
                            out=s_t[:], in0=iota_f[:],
                            scalar1=posT[:, col:col + 1], scalar2=None,
                            op0=mybir.AluOpType.is_equal)
                        lhs = fb[:, (col - c0) * 2 * C:(col - c0 + 1) * 2 * C]
                        nc.tensor.matmul(
                            out=ps[:], lhsT=lhs,
                            rhs=s_t[:],
                            start=(k == 0), stop=(k == nck - 1))
                    off = (pr - p0) * TILE
                    eng = nc.vector.tensor_copy if pr % 5 == 0 else (
                        lambda out, in_: nc.scalar.copy(out=out, in_=in_))
                    eng(out=stg[:, off:off + TILE], in_=ps[:])
                # two out DMAs: A half + B half, each contiguous (<=8KB runs)
                wa = (p1 - p0) * TILE
                a0 = p0 * TILE
                nc.sync.dma_start(out=out_d[0:C, a0:a0 + wa],
                                  in_=stg[:64, :wa])
                wb = min(BCELL - a0, wa)
                if wb > 0:
                    nc.sync.dma_start(
                        out=out_d[0:C, ACELL + a0:ACELL + a0 + wb],
                        in_=stg[64:, :wb])
                while (emitted_maps < len(map_groups)
                       and emitted_maps <= g // mg_every):
                    emit_map_group(*next(mg_iter))
                    emitted_maps += 1
            for mg in mg_iter:
                emit_map_group(*mg)

    nc.finalize()
    return nc


def _host_prep(voxel_features, coords, map_fm):
    """Shard points by core, build feature/pos tables (host index work only)."""
    vf = np.ascontiguousarray(np.asarray(voxel_features), dtype=np.float32)
    cd = np.asarray(coords)
    mf = np.asarray(map_fm)
    if mf.ndim == 5:
        mf = np.squeeze(mf, 3)
    mf = np.ascontiguousarray(mf, dtype=np.float32)

    b = cd[:, 0].astype(np.int64)
    y = cd[:, 2].astype(np.int64)
    x = cd[:, 3].astype(np.int64)
    valid = (b >= 0) & (b < B) & (y >= 0) & (y < NY) & (x >= 0) & (x < NX)
    b, y, x = b[valid], y[valid], x[valid]
    vfv = vf[valid]

    half = (y >= NYH).astype(np.int64)
    core = b * 2 + half
    lcell = (y - half * NYH) * NX + x
    t = lcell // TILE          # 512-cell tile id
    pos = lcell - t * TILE     # position within tile (= matmul column)
    pair = t % NP              # tile j pairs with tile j+NP
    blk = t // NP              # channel block within the pair

    key = core * NP + pair
    order = np.argsort(key, kind="stable")
    ks = key[order]
    counts = np.bincount(ks, minlength=NCORE * NP)
    kmax = counts.reshape(NCORE, NP).max(axis=0)
    chunks = np.maximum((kmax + CAP - 1) // CAP, 1)
    for g in range(0, NP, SG):
        need = int(chunks[g:g + SG].sum())
        if need > FB:
            raise ValueError("pair group needs %d cols > FB=%d" % (need, FB))
    ncols = int(chunks.sum())
    colbase = np.concatenate([[0], np.cumsum(chunks)]).astype(np.int64)

    starts = np.concatenate([[0], np.cumsum(counts)]).astype(np.int64)
    rank = np.arange(len(ks), dtype=np.int64) - starts[ks]

    co = core[order]
    po = pair[order]
    bo = blk[order]
    colo = colbase[po] + rank // CAP
    slot = rank % CAP

    # exact fp32 = hi + lo split, both parts e8m11 (fp32r) representable
    vo = vfv[order]
    hi = (vo.view(np.uint32) & np.uint32(0xFFFFF000)).view(np.float32)
    lo = vo - hi

    feat = np.zeros((NCORE, 2 * CAP, ncols, 2 * C), np.float32)
    post = np.full((NCORE, 2 * CAP, ncols), -1.0, np.float32)
    ccol = bo[:, None] * C + np.arange(C)[None, :]
    feat[co[:, None], slot[:, None], colo[:, None], ccol] = hi
    feat[co[:, None], (CAP + slot)[:, None], colo[:, None], ccol] = lo
    posf = pos[order].astype(np.float32)
    post[co, slot, colo] = posf
    post[co, CAP + slot, colo] = posf

    maps = []
    for core_id in range(NCORE):
        bb, hh = core_id // 2, core_id % 2
        maps.append(np.ascontiguousarray(
            mf[bb, :, hh * NYH:(hh + 1) * NYH, :]))
    return feat, post, maps, ncols, chunks


def kernel(voxel_features, coords, batch_size=None, map_fm=None,
           trace=False, _return_results=False):
    from concourse.bass_utils import run_bass_kernel_spmd

    feat, post, maps, ncols, chunks = _host_prep(
        voxel_features, coords, map_fm)

    ckey = (ncols, tuple(int(c) for c in chunks))
    if ckey not in _prog_cache:
        _prog_cache.clear()
        _prog_cache[ckey] = _build_program(ncols, chunks)
    nc = _prog_cache[ckey]

    in_maps = [
        {"feat": feat[i].reshape(2 * CAP, -1), "post": post[i],
         "mapin": maps[i]}
        for i in range(NCORE)
    ]
    res = run_bass_kernel_spmd(nc, in_maps, list(range(NCORE)), trace=trace)

    out = np.empty((B, C + CM, NY, NX), np.float32)
    for core_id in range(NCORE):
        bb, hh = core_id // 2, core_id % 2
        out[bb, :, hh * NYH:(hh + 1) * NYH, :] = (
            res.results[core_id]["out"].reshape(C + CM, NYH, NX))
    if _return_results:
        return out, res
    return out


# revision 29
# speedup vs baseline: 7.5752x; 1.0929x over previous
"""PointPillarsScatter Trainium2 kernel.

Reference op:
  canvas[b*NY*NX + y*NX + x] = voxel_features[p]        (scatter-set, 64 ch)
  out[:, :64]  = canvas -> [B, 64, NY, NX]
  out[:, 64:]  = transpose(map_fm, (0, 3, 2, 1))        (16 ch)

Strategy (8 NeuronCores, SPMD):
  core = batch*2 + y_half  (4 batches x 2 halves of NY=496 -> NYH=248 rows).

  Scatter is computed as a one-hot matmul on the TensorEngine:
    out[128ch', 512cells] = featT[128slots, 128ch'].T @ S[128slots, 512]
  where S[s, n] = (pos[s] == n) is built on the VectorEngine with
  iota + is_equal, and ch' packs the 64 channels of TWO 512-cell tiles
  (tile j -> psum partitions 0:64, tile j+105 -> 64:128, so each
  partition half maps to a contiguous DRAM range).  This fuses
  zero-fill + scatter + transpose into one PE op per 1024 cells.

  The matmul runs in fp32r mode (1 column/cycle vs 4 for fp32). fp32r is
  e8m11 (lossy), so each feature value v is split exactly into
  v = hi + lo, both parts e8m11-representable (hi = mantissa truncated
  to 11 bits, lo = v - hi has <= 12 significant bits). hi parts occupy
  slots 0..63, lo parts slots 64..127 with identical S rows, so a single
  K=128 matmul reconstructs v exactly in fp32 PSUM.

  map_fm is transposed with PE transpose (identity) in [<=128, 128]
  blocks, staged per output y-row so every DMA moves 1728B contiguous
  runs.

Host side only computes index tables + shards inputs (per the sharding
hint: route points by coords to their core); all FP math runs on device.
"""

import sys

for _p in ("/opt/trn_rl_repo",):
    if _p not in sys.path:
        sys.path.insert(0, _p)

import numpy as np

# problem constants (hardcoded per contract)
B, NPTS, C, NY, NX, CM = 4, 48000, 64, 496, 432, 16
NYH = NY // 2            # 248 rows per core
NCORE = 8
NCELL = NYH * NX         # 107136 cells per core
TILE = 512               # cells per channel-block
PAIR = 2 * TILE          # cells per matmul (two 64ch blocks -> M=128)
NT = (NCELL + TILE - 1) // TILE          # 210 tiles (last has 128 cells)
NP = (NT + 1) // 2                       # 105 pairs: tile j with tile j+NP
ACELL = NP * TILE                        # 53760 cells in the A half
BCELL = NCELL - ACELL                    # 53376 cells in the B half
CAP = 64                 # points per pair-column (x2 slots for hi/lo)
FB = 16                  # pair-columns per feature DMA load
SG = 8                   # pairs per SBUF staging buffer / out DMA
YB = 8                   # map y rows per transpose block ( YB*CM = 128 )
NYB = NYH // YB          # 31 y-blocks
XCH = [(0, 128), (128, 128), (256, 128), (384, 48)]   # x chunks of NX=432

_prog_cache = {}


def _build_program(ncols, chunks):
    """Build the SPMD Bass program (identical for all 8 cores)."""
    from concourse import bacc, mybir, tile
    from concourse.masks import make_identity

    f32 = mybir.dt.float32
    f32r = mybir.dt.float32r
    i32 = mybir.dt.int32

    nc = bacc.Bacc(trn_type="TRN2", target_bir_lowering=False)

    # slot-major layout: partition s reads one contiguous 4KB run per load
    feat_d = nc.dram_tensor("feat", [2 * CAP, ncols * 2 * C], f32r,
                            kind="ExternalInput")
    post_d = nc.dram_tensor("post", [2 * CAP, ncols], f32, kind="ExternalInput")
    map_d = nc.dram_tensor("mapin", [NX, NYH, CM], f32, kind="ExternalInput")
    out_d = nc.dram_tensor("out", [C + CM, NCELL], f32, kind="ExternalOutput")

    # column index of each (pair, chunk)
    colbase = np.concatenate([[0], np.cumsum(chunks)]).astype(np.int64)

    # map work: one transpose+store group per y-block
    map_groups = list(range(NYB))
    mg_iter = iter(map_groups)
    n_sc_groups = (NP + SG - 1) // SG

    with tile.TileContext(nc) as tc:
        with (
            tc.tile_pool(name="const", bufs=1) as cpool,
            tc.tile_pool(name="fpool", bufs=2) as fpool,
            tc.tile_pool(name="spool", bufs=4) as spool,
            tc.tile_pool(name="stg", bufs=2) as stpool,
            tc.tile_pool(name="mstg", bufs=3) as mstpool,
            tc.tile_pool(name="mtin", bufs=1) as mtpool,
            tc.tile_pool(name="pscat", bufs=4, space="PSUM") as pspool,
            tc.tile_pool(name="pmap", bufs=3, space="PSUM") as pmpool,
        ):
            # constants
            iota_i = cpool.tile([2 * CAP, TILE], i32)
            nc.gpsimd.iota(iota_i[:], pattern=[[1, TILE]], base=0,
                           channel_multiplier=0)
            iota_f = cpool.tile([2 * CAP, TILE], f32)
            nc.gpsimd.tensor_copy(iota_f[:], iota_i[:])
            ident = cpool.tile([128, 128], f32)
            make_identity(nc, ident[:])
            posT = cpool.tile([2 * CAP, ncols], f32)
            nc.sync.dma_start(out=posT[:], in_=post_d[:])

            # preload the whole map input: 4 DMAs with 15.5KB runs
            mts = []
            for x0, w in XCH:
                mt = mtpool.tile([128, NYB * YB * CM], f32, tag="mt%d" % x0)
                nc.scalar.dma_start(
                    out=mt[:w, :],
                    in_=map_d[x0:x0 + w, :, :].rearrange("x y c -> x (y c)"))
                mts.append(mt)
            out_map = out_d[C:C + CM, :].rearrange("c (yy xx) -> c yy xx",
                                                   xx=NX)

            def emit_map_group(k):
                pm = pmpool.tile([128, NX], f32)
                for xi, (x0, w) in enumerate(XCH):
                    nc.tensor.transpose(
                        out=pm[:, x0:x0 + w],
                        in_=mts[xi][:w, (k * YB * CM):((k + 1) * YB * CM)],
                        identity=ident[:w, :w])
                ms = mstpool.tile([128, NX], f32)
                nc.scalar.copy(out=ms[:], in_=pm[:])
                # one DMA per y-block: runs of NX*4 = 1728B
                dst = out_map[:, k * YB:(k + 1) * YB, :]
                nc.scalar.dma_start(
                    out=dst.rearrange("c dy x -> dy c x"), in_=ms[:])

            # scatter loop over groups of SG pairs
            emitted_maps = 0
            for g in range(n_sc_groups):
                p0 = g * SG
                p1 = min(p0 + SG, NP)
                c0 = int(colbase[p0])
                c1 = int(colbase[p1])
                fb = fpool.tile([2 * CAP, FB * 2 * C], f32r, tag="fb")
                assert c1 - c0 <= FB, (c0, c1)
                nc.sync.dma_start(
                    out=fb[:, :(c1 - c0) * 2 * C],
                    in_=feat_d[:, c0 * 2 * C:c1 * 2 * C])
                stg = stpool.tile([128, SG * TILE], f32)
                for pr in range(p0, p1):
                    # block A = tile pr (always 512 cells),
                    # block B = tile NP+pr (last one has 128 cells)
                    nbb = min(TILE, max(0, NCELL - (NP + pr) * TILE))
                    ps = pspool.tile([128, TILE], f32)
                    nck = int(chunks[pr])
                    for k in range(nck):
                        col = int(colbase[pr]) + k
                        s_t = spool.tile([2 * CAP, TILE], f32r)
                        nc.vector.tensor_scalar(
                            out=s_t[:], in0=iota_f[:],
                            scalar1=posT[:, col:col + 1], scalar2=None,
                            op0=mybir.AluOpType.is_equal)
                        lhs = fb[:, (col - c0) * 2 * C:(col - c0 + 1) * 2 * C]
                        nc.tensor.matmul(
                            out=ps[:], lhsT=lhs,
                            rhs=s_t[:],
                            start=(k == 0), stop=(k == nck - 1))
                    off = (pr - p0) * TILE
                    eng = nc.vector.tensor_copy if pr % 5 == 0 else (
                        lambda out, in_: nc.scalar.copy(out=out, in_=in_))
                    eng(out=stg[:, off:off + TILE], in_=ps[:])
                # two out DMAs: A half + B half, each contiguous (<=8KB runs)
                wa = (p1 - p0) * TILE
                a0 = p0 * TILE
                nc.sync.dma_start(out=out_d[0:C, a0:a0 + wa],
                                  in_=stg[:64, :wa])
                wb = min(BCELL - a0, wa)
                if wb > 0:
                    nc.sync.dma_start(
                        out=out_d[0:C, ACELL + a0:ACELL + a0 + wb],
                        in_=stg[64:, :wb])
                while (emitted_maps < len(map_groups)
                       and emitted_maps * n_sc_groups <= (g + 1) * NYB):
                    emit_map_group(next(mg_iter))
                    emitted_maps += 1
            for mg in mg_iter:
                emit_map_group(mg)

    nc.finalize()
    return nc


def _host_prep(voxel_features, coords, map_fm):
    """Shard points by core, build feature/pos tables (host index work only)."""
    vf = np.ascontiguousarray(np.asarray(voxel_features), dtype=np.float32)
    cd = np.asarray(coords)
    mf = np.asarray(map_fm)
    if mf.ndim == 5:
        mf = np.squeeze(mf, 3)
    mf = np.ascontiguousarray(mf, dtype=np.float32)

    b = cd[:, 0].astype(np.int64)
    y = cd[:, 2].astype(np.int64)
    x = cd[:, 3].astype(np.int64)
    valid = (b >= 0) & (b < B) & (y >= 0) & (y < NY) & (x >= 0) & (x < NX)
    b, y, x = b[valid], y[valid], x[valid]
    vfv = vf[valid]

    half = (y >= NYH).astype(np.int64)
    core = b * 2 + half
    lcell = (y - half * NYH) * NX + x
    t = lcell // TILE          # 512-cell tile id
    pos = lcell - t * TILE     # position within tile (= matmul column)
    pair = t % NP              # tile j pairs with tile j+NP
    blk = t // NP              # channel block within the pair

    key = core * NP + pair
    order = np.argsort(key, kind="stable")
    ks = key[order]
    counts = np.bincount(ks, minlength=NCORE * NP)
    kmax = counts.reshape(NCORE, NP).max(axis=0)
    chunks = np.maximum((kmax + CAP - 1) // CAP, 1)
    for g in range(0, NP, SG):
        need = int(chunks[g:g + SG].sum())
        if need > FB:
            raise ValueError("pair group needs %d cols > FB=%d" % (need, FB))
    ncols = int(chunks.sum())
    colbase = np.concatenate([[0], np.cumsum(chunks)]).astype(np.int64)

    starts = np.concatenate([[0], np.cumsum(counts)]).astype(np.int64)
    rank = np.arange(len(ks), dtype=np.int64) - starts[ks]

    co = core[order]
    po = pair[order]
    bo = blk[order]
    colo = colbase[po] + rank // CAP
    slot = rank % CAP

    # exact fp32 = hi + lo split, both parts e8m11 (fp32r) representable
    vo = vfv[order]
    hi = (vo.view(np.uint32) & np.uint32(0xFFFFF000)).view(np.float32)
    lo = vo - hi

    feat = np.zeros((NCORE, 2 * CAP, ncols, 2 * C), np.float32)
    post = np.full((NCORE, 2 * CAP, ncols), -1.0, np.float32)
    ccol = bo[:, None] * C + np.arange(C)[None, :]
    feat[co[:, None], slot[:, None], colo[:, None], ccol] = hi
    feat[co[:, None], (CAP + slot)[:, None], colo[:, None], ccol] = lo
    posf = pos[order].astype(np.float32)
    post[co, slot, colo] = posf
    post[co, CAP + slot, colo] = posf

    maps = []
    for core_id in range(NCORE):
        bb, hh = core_id // 2, core_id % 2
        maps.append(np.ascontiguousarray(
            mf[bb, :, hh * NYH:(hh + 1) * NYH, :]))
    return feat, post, maps, ncols, chunks


def kernel(voxel_features, coords, batch_size=None, map_fm=None,
           trace=False, _return_results=False):
    from concourse.bass_utils import run_bass_kernel_spmd

    feat, post, maps, ncols, chunks = _host_prep(
        voxel_features, coords, map_fm)

    ckey = (ncols, tuple(int(c) for c in chunks))
    if ckey not in _prog_cache:
        _prog_cache.clear()
        _prog_cache[ckey] = _build_program(ncols, chunks)
    nc = _prog_cache[ckey]

    in_maps = [
        {"feat": feat[i].reshape(2 * CAP, -1), "post": post[i],
         "mapin": maps[i]}
        for i in range(NCORE)
    ]
    res = run_bass_kernel_spmd(nc, in_maps, list(range(NCORE)), trace=trace)

    out = np.empty((B, C + CM, NY, NX), np.float32)
    for core_id in range(NCORE):
        bb, hh = core_id // 2, core_id % 2
        out[bb, :, hh * NYH:(hh + 1) * NYH, :] = (
            res.results[core_id]["out"].reshape(C + CM, NYH, NX))
    if _return_results:
        return out, res
    return out
